# revision 1
# baseline (speedup 1.0000x reference)
"""TRN2 Bass kernel for nn_Attention_56392920596865.

Structure exploited (B=4, S=2048, D=1024, H=16, HD=64):
  - The "buggy head shuffle" maps chunk (b, s, h) -> shuffled batch b' = s//512,
    so attention for shuffled batch b' only consumes projected rows from input
    sequence window s in [512b', 512(b'+1)), all input batches. Each core
    (bp = c//2 over shuffled batch, qh = c%2 over query halves) computes its own
    Q/K/V projections locally -> no collectives.
  - The second shuffle gives each core exactly 2 of the 16 mh feature blocks for
    ALL output rows -> each core computes a partial o = mh[:, blk] @ W_o[:, blk]^T
    over all 8192 rows and the host sums the 8 partials.
  - All matmuls run as float32r (full PE speed at free-dim >= 256, ~1.5e-4 rel
    err vs fp32; end-to-end ~4.8e-4).
  - Shuffle layout uses a consistent column permutation col' = (h>>2)*nsig + sigma
    of the shuffled k'/q' index so every psum scatter-evict is contiguous; the
    permutation cancels inside the attention contraction sums.

Per-core phases (one Tile program; phases overlap via emission interleaving):
  1/2. K''^T and Q''^T via projection matmuls with shuffle-scatter psum evicts
  3.   S^T = K''^T.T @ Q''^T (scores transposed), ACT exp((1/32) s) -> expS
  4.   V projection -> V''^T scatter -> PE-transpose -> V'' (k'-natural)
  5.   Z = expS-column matmuls; rep = (expS.T @ V'') / Z written (d,parity)-
       interleaved per qs pair; PE-transpose pairs -> repT2 [(dh,delta), h', r0, m]
  6.   (interleaved with 5) o_part row tiles = repT2 K=128 matmuls against
       host-row-interleaved W_o^T slice; host unscrambles the (h', r0, b, hi)
       row permutation: s = hi*64 + r0*16 + h'.
"""
import sys
import numpy as np

try:
    import concourse.bass  # noqa: F401
except ImportError:
    sys.path.insert(0, "/opt/trn_rl_repo")

B, S, D, H, HD = 4, 2048, 1024, 16, 64

_CACHE = {}


def _build_program():
    from contextlib import ExitStack

    import concourse.mybir as mybir
    import concourse.tile as tile
    from concourse import bacc

    F32 = mybir.dt.float32
    F32R = mybir.dt.float32r
    AFT = mybir.ActivationFunctionType

    nc = bacc.Bacc(None, target_bir_lowering=False, debug=False)

    with tile.TileContext(nc) as tc:
        with tc.tile_pool(name="dram", bufs=1, space="DRAM") as dram:
            kT = dram.tile([1024, 2048], F32R, kind="ExternalInput", name="kT", uniquify=False)
            qT = dram.tile([1024, 1024], F32R, kind="ExternalInput", name="qT", uniquify=False)
            vT = dram.tile([1024, 2048], F32R, kind="ExternalInput", name="vT", uniquify=False)
            wkT = dram.tile([1024, 1024], F32R, kind="ExternalInput", name="wkT", uniquify=False)
            wqT = dram.tile([1024, 1024], F32R, kind="ExternalInput", name="wqT", uniquify=False)
            wvT = dram.tile([1024, 1024], F32R, kind="ExternalInput", name="wvT", uniquify=False)
            woTa = dram.tile([128, 1024], F32R, kind="ExternalInput", name="woTa", uniquify=False)
            ones1 = dram.tile([128, 4], F32, kind="ExternalInput", name="ones1", uniquify=False)
            ident = dram.tile([128, 128], F32R, kind="ExternalInput", name="ident", uniquify=False)
            o_part = dram.tile([8192, 1024], F32, kind="ExternalOutput", name="o_part", uniquify=False)

            # x blocks go on the SP HWDGE queue; W chunks on the ACT HWDGE
            # queue so neither stream head-of-line-blocks the other.
            def load_w_full(pool, w_dram, nm):
                w_sb = pool.tile([128, 8, 1024], F32R, name=nm, tag="wfull")
                w_r = w_dram.rearrange("(t p) c -> p t c", p=128)
                for j in range(8):
                    nc.scalar.dma_start(w_sb[:, :, j * 128:(j + 1) * 128],
                                        w_r[:, :, j * 128:(j + 1) * 128])
                return w_sb

            def scatter_evict(dst_fn, ps, j, gcol0, width, nsig):
                seg = min(nsig, width)
                for hh in (0, 1):
                    h = 2 * j + hh
                    for s_off in range(0, width, seg):
                        gcol = gcol0 + s_off
                        b = gcol // nsig
                        hp = 4 * (h & 3) + b
                        c0 = (h >> 2) * nsig + (gcol % nsig)
                        dst = dst_fn(hp)[64 * (hp & 1):64 * (hp & 1) + 64, c0:c0 + seg]
                        srcp = ps[64 * hh:64 * hh + 64, s_off:s_off + seg]
                        if hh == 0:
                            nc.vector.tensor_copy(dst, srcp)
                        else:
                            nc.scalar.copy(dst, srcp)

            def proj_scatter(dst_fn, x_dram, nsig, blocks, w_sb, stg, psp):
                """Project x window by W^T; scatter-evict into shuffled-
                transposed dst. blocks = list of (col0, width)."""
                x_r = x_dram.rearrange("(t p) c -> p t c", p=128)
                for c0b, wb in blocks:
                    x_sb = stg.tile([128, 8, 512], F32R, name="x_sb", tag="x_sb",
                                    padded_shape=[128, 8, 512])
                    nc.sync.dma_start(x_sb[:, :, 0:wb], x_r[:, :, c0b:c0b + wb])
                    for j in range(8):
                        ps = psp.tile([128, 512], F32, name="ps", tag="ps")
                        for t in range(8):
                            nc.tensor.matmul(ps[:, 0:wb], w_sb[:, t, j * 128:(j + 1) * 128],
                                             x_sb[:, t, 0:wb], start=(t == 0), stop=(t == 7))
                        scatter_evict(dst_fn, ps[:, 0:wb], j, c0b, wb, nsig)

            stkKQ = ExitStack()
            pK = stkKQ.enter_context(tc.tile_pool(name="pK", bufs=1))
            K2T = pK.tile([128, 8, 2048], F32R, name="K2T")
            pQ = stkKQ.enter_context(tc.tile_pool(name="pQ", bufs=1))
            Q2T = pQ.tile([128, 8, 1024], F32R, name="Q2T")

            # phase 1 + 2: K and Q projections (full-W tiles, double-buffered)
            with tc.tile_pool(name="pW", bufs=2) as pW, \
                 tc.tile_pool(name="stp", bufs=2) as stp, \
                 tc.tile_pool(name="pspA", bufs=8, space="PSUM") as pspA:
                w_k = load_w_full(pW, wkT, "w_k")
                proj_scatter(lambda hp: K2T[:, hp >> 1, :], kT, 512,
                             [(0, 256), (256, 256), (512, 512), (1024, 512), (1536, 512)],
                             w_sb=w_k, stg=stp, psp=pspA)
                w_q = load_w_full(pW, wqT, "w_q")
                proj_scatter(lambda hp: Q2T[:, hp >> 1, :], qT, 256,
                             [(0, 512), (512, 512)], w_sb=w_q, stg=stp, psp=pspA)

            # phase 3: scores^T + exp
            stkE = ExitStack()
            pE = stkE.enter_context(tc.tile_pool(name="pE", bufs=1, side="right"))
            expS = pE.tile([128, 16, 1024], F32R, name="expS")
            with tc.tile_pool(name="ps_s", bufs=8, space="PSUM") as pss:
                for qb in range(2):
                    for kt in range(16):
                        ps = pss.tile([128, 512], F32, name="ps_sc", tag="ps_sc")
                        for t in range(8):
                            nc.tensor.matmul(ps[:], K2T[:, t, kt * 128:(kt + 1) * 128],
                                             Q2T[:, t, qb * 512:(qb + 1) * 512],
                                             start=(t == 0), stop=(t == 7))
                        nc.scalar.activation(expS[:, kt, qb * 512:(qb + 1) * 512], ps[:],
                                             AFT.Exp, scale=1.0 / 32.0)
            stkKQ.close()

            # phase 4: V projection -> V''T -> PE-transpose -> V'' natural
            stkV = ExitStack()
            pV = stkV.enter_context(tc.tile_pool(name="pV", bufs=1))
            V2 = pV.tile([128, 16, 1024], F32R, name="V2")
            stkI = ExitStack()
            cpool = stkI.enter_context(tc.tile_pool(name="cpool", bufs=1, side="right"))
            id_sb = cpool.tile([128, 128], F32R, name="id_sb")
            nc.scalar.dma_start(id_sb[:], ident[:])
            with ExitStack() as ctxv:
                v2t_pool = ctxv.enter_context(tc.tile_pool(name="v2t", bufs=4))
                vstg = ctxv.enter_context(tc.tile_pool(name="vstg", bufs=2))
                wvp = ctxv.enter_context(tc.tile_pool(name="wvp", bufs=3))
                pspV = ctxv.enter_context(tc.tile_pool(name="pspV", bufs=4, space="PSUM"))
                pst_pool = ctxv.enter_context(tc.tile_pool(name="ps_t", bufs=4, space="PSUM"))
                v2t_tiles = {}

                def v_dst(hp):
                    tau = hp >> 1
                    if tau not in v2t_tiles:
                        v2t_tiles[tau] = v2t_pool.tile([128, 2048], F32R,
                                                       name=f"v2t_{tau}", tag="v2t")
                    return v2t_tiles[tau]

                v_r = vT.rearrange("(t p) c -> p t c", p=128)
                wv_r = wvT.rearrange("(t p) c -> p t c", p=128)
                for pair in range(2):
                    x_pair = []
                    for bb in (2 * pair, 2 * pair + 1):
                        x_sb = vstg.tile([128, 8, 512], F32R, name="x_sb", tag="vx_sb")
                        nc.sync.dma_start(x_sb[:], v_r[:, :, bb * 512:(bb + 1) * 512])
                        x_pair.append(x_sb)
                    for j in range(8):
                        w_j = wvp.tile([128, 8, 128], F32R, name="w_j", tag="w_j")
                        nc.sync.dma_start(w_j[:], wv_r[:, :, j * 128:(j + 1) * 128])
                        for bi, bb in enumerate((2 * pair, 2 * pair + 1)):
                            ps = pspV.tile([128, 512], F32, name="ps", tag="vps")
                            for t in range(8):
                                nc.tensor.matmul(ps[:], w_j[:, t, :], x_pair[bi][:, t, :],
                                                 start=(t == 0), stop=(t == 7))
                            scatter_evict(v_dst, ps[:], j, bb * 512, 512, 512)
                    taus = (0, 2, 4, 6) if pair == 0 else (1, 3, 5, 7)
                    for tau in taus:
                        vt = v2t_tiles.pop(tau)
                        for kt in range(16):
                            pst = pst_pool.tile([128, 128], F32R, name="pst", tag="pst")
                            nc.tensor.transpose(pst[:], vt[:, kt * 128:(kt + 1) * 128], id_sb[:])
                            if kt % 2 == 0:
                                nc.vector.tensor_copy(V2[:, kt, tau * 128:(tau + 1) * 128], pst[:])
                            else:
                                nc.scalar.copy(V2[:, kt, tau * 128:(tau + 1) * 128], pst[:])

            # phase 5: Z + AV -> rep (SBUF) -> PE-transpose -> repT (SBUF)
            stkR = ExitStack()
            pR = stkR.enter_context(tc.tile_pool(name="pR", bufs=1))
            repT2 = pR.tile([128, 16, 4, 128], F32R, name="repT2")
            scratch = stkR.enter_context(tc.tile_pool(name="scratch", bufs=4))
            wop = stkR.enter_context(tc.tile_pool(name="wop", bufs=1))
            wo_a = wop.tile([128, 1024], F32R, name="wo_a")
            nc.scalar.dma_start(wo_a[:], woTa[:])
            with ExitStack() as ctxa:
                cp2 = ctxa.enter_context(tc.tile_pool(name="cp2", bufs=1))
                ones_sb = cp2.tile([128, 4], F32, name="ones_sb")
                nc.scalar.dma_start(ones_sb[:], ones1[:])
                rzp = ctxa.enter_context(tc.tile_pool(name="rzp", bufs=2))
                pz = ctxa.enter_context(tc.tile_pool(name="pz", bufs=1, space="PSUM"))
                pav = ctxa.enter_context(tc.tile_pool(name="pav", bufs=2, space="PSUM"))
                prt = ctxa.enter_context(tc.tile_pool(name="prt", bufs=3, space="PSUM"))
                pso = ctxa.enter_context(tc.tile_pool(name="pso", bufs=2, space="PSUM"))

                def emit_phase6_r0(r0, h_lo=0, h_hi=16):
                    # needs repT2[:, :, r0, :] = qs subtiles 2r0 (dh=0) and 2r0+1 (dh=1)
                    for hp16 in range(h_lo, h_hi):
                        ost = scratch.tile([128, 1024], F32, name="ost", tag="scr")
                        lhsT = repT2[:, hp16, r0, :]
                        row0 = hp16 * 512 + r0 * 128
                        for half in range(2):
                            po = pso.tile([128, 512], F32, name="po", tag="po")
                            nc.tensor.matmul(po[:], lhsT, wo_a[:, half * 512:(half + 1) * 512],
                                             start=True, stop=True)
                            if half == 0:
                                nc.scalar.copy(ost[:, 0:512], po[:])
                            else:
                                nc.vector.tensor_copy(ost[:, 512:1024], po[:])
                            nc.sync.dma_start(
                                o_part[row0:row0 + 128, half * 512:(half + 1) * 512],
                                ost[:, half * 512:(half + 1) * 512])

                pairs, pending = {}, None

                def emit_pair_transposes(r0q):
                    rp = pairs.pop(r0q)
                    for hp16 in range(16):
                        prt_t = prt.tile([128, 128], F32R, name="prt_t", tag="prt_t")
                        nc.tensor.transpose(prt_t[:], rp[:, hp16 * 128:(hp16 + 1) * 128],
                                            id_sb[:])
                        nc.vector.tensor_copy(repT2[:, hp16, r0q, :], prt_t[:])

                for qs in range(8):
                    if qs % 2 == 0 and pending is not None:
                        emit_phase6_r0(pending, 0, 4)
                    zp = pz.tile([128, 4], F32, name="zp", tag="zp")
                    for kt in range(16):
                        nc.tensor.matmul(zp[:], expS[:, kt, qs * 128:(qs + 1) * 128].bitcast(F32),
                                         ones_sb[:], start=(kt == 0), stop=(kt == 15))
                    rz = rzp.tile([128, 1], F32, name="rz", tag="rz")
                    nc.vector.reciprocal(rz[:], zp[:, 0:1])
                    par, r0q = qs & 1, qs >> 1
                    if par == 0:
                        pairs[r0q] = scratch.tile([128, 2048], F32R, name="rep_pair", tag="scr")
                    rep_pair = pairs[r0q]
                    for df in range(2):
                        pa = pav.tile([128, 512], F32, name="pa", tag="pa")
                        for kt in range(16):
                            nc.tensor.matmul(pa[:], expS[:, kt, qs * 128:(qs + 1) * 128],
                                             V2[:, kt, df * 512:(df + 1) * 512],
                                             start=(kt == 0), stop=(kt == 15))
                        # interleaved dest: col = d*2 + parity
                        nc.scalar.activation(
                            rep_pair[:, df * 1024 + par:df * 1024 + par + 1023:2], pa[:],
                            AFT.Copy, scale=rz[:])
                        if df == 0 and par == 0 and pending is not None:
                            emit_phase6_r0(pending, 4, 10)
                    if par == 0 and pending is not None:
                        emit_phase6_r0(pending, 10, 16)
                        pending = None
                    if par == 1:
                        emit_pair_transposes(r0q)
                        pending = r0q
                emit_phase6_r0(pending)
            stkI.close()
            stkE.close()

            stkR.close()
            stkV.close()

    nc.compile()
    return nc


def _host_inputs(k, q, v, W_k, W_q, W_v, W_o):
    """Per-core input maps. Core c: bp = c//2 (shuffled batch), qh = c%2."""
    f32 = np.float32
    W_kT = np.ascontiguousarray(W_k.T, dtype=f32)
    W_qT = np.ascontiguousarray(W_q.T, dtype=f32)
    W_vT = np.ascontiguousarray(W_v.T, dtype=f32)
    W_oT = np.ascontiguousarray(W_o.T, dtype=f32)
    ones = np.ones((128, 4), dtype=f32)
    ident = np.eye(128, dtype=f32)
    in_maps = []
    for c in range(8):
        bp, qh = c // 2, c % 2
        kw = k[:, 512 * bp:512 * (bp + 1), :].reshape(2048, 1024)
        vw = v[:, 512 * bp:512 * (bp + 1), :].reshape(2048, 1024)
        qw = q[:, 512 * bp + 256 * qh:512 * bp + 256 * (qh + 1), :].reshape(1024, 1024)
        h0 = 4 * bp + 2 * qh
        wo_nat = W_oT[h0 * 64:h0 * 64 + 128, :]
        wo_nat = np.ascontiguousarray(
            wo_nat.reshape(2, 64, 1024).transpose(1, 0, 2).reshape(128, 1024))
        in_maps.append({
            "kT": np.ascontiguousarray(kw.T, dtype=f32),
            "vT": np.ascontiguousarray(vw.T, dtype=f32),
            "qT": np.ascontiguousarray(qw.T, dtype=f32),
            "wkT": W_kT, "wqT": W_qT, "wvT": W_vT,
            "woTa": wo_nat,
            "ones1": ones, "ident": ident,
        })
    return in_maps


def kernel(k, q, v, W_k, W_q, W_v, W_o, _want_trace=False):
    from concourse.bass_utils import run_bass_kernel_spmd

    if "nc" not in _CACHE:
        _CACHE["nc"] = _build_program()
    nc = _CACHE["nc"]

    in_maps = _host_inputs(np.asarray(k), np.asarray(q), np.asarray(v),
                           np.asarray(W_k), np.asarray(W_q), np.asarray(W_v),
                           np.asarray(W_o))
    res = run_bass_kernel_spmd(nc, in_maps, core_ids=list(range(8)),
                               trace=_want_trace)
    out = np.zeros((8192, 1024), dtype=np.float64)
    for r in res.results:
        out += r["o_part"].astype(np.float64)
    # rows are (h', r0, b, hi); real s = hi*64 + r0*16 + h'
    out = out.astype(np.float32).reshape(16, 4, 4, 32, D).transpose(2, 3, 1, 0, 4).reshape(B, S, D)
    if _want_trace:
        _CACHE["last_result"] = res
    return out



# revision 2
# speedup vs baseline: 1.0619x; 1.0619x over previous
"""TRN2 Bass kernel for nn_Attention_56392920596865.

Structure exploited (B=4, S=2048, D=1024, H=16, HD=64):
  - The "buggy head shuffle" maps chunk (b, s, h) -> shuffled batch b' = s//512,
    so attention for shuffled batch b' only consumes projected rows from input
    sequence window s in [512b', 512(b'+1)), all input batches. Each core
    (bp = c//2 over shuffled batch, qh = c%2 over query halves) computes its own
    Q/K/V projections locally -> no collectives.
  - The second shuffle gives each core exactly 2 of the 16 mh feature blocks for
    ALL output rows -> each core computes a partial o = mh[:, blk] @ W_o[:, blk]^T
    over all 8192 rows and the host sums the 8 partials.
  - Everything runs in fp16 (matmul 1 cycle/row like f32r, but half the DMA /
    SBUF footprint and 1.0 c/r transposes; end-to-end abs-max rel err ~1e-3).
  - Shuffled tensors use a consistent bijection of the 1024 features onto
    (partition p, tau): for original feature h*64+hd (h = 2j+hh) and input
    batch b:  p = 64*hh + hd, tau = (j&1)*4 + b, key column = (j>>1)*nsig + s.
    With this choice a projection psum tile [128, w] (partitions = (hh, hd))
    evicts with ONE full-height copy per (j, b) -> half the ACT/DVE time of a
    split-eviction layout. The key-column permutation cancels inside the
    attention contraction.

Per-core phases (one Tile program; phases overlap via emission interleaving):
  1/2. K''^T and Q''^T via projection matmuls with merged shuffle-scatter evicts
  3.   S^T = K''^T.T @ Q''^T (scores transposed), ACT exp((1/32) s) -> expS
  4.   V projection -> V''^T scatter -> PE-transpose (fp16) -> V'' (k-natural)
  5.   Z = expS-column matmuls; rep = (expS.T @ V'') / Z written (d,parity)-
       interleaved per qs pair; PE-transpose pairs -> repT2 [(dh,delta,par), ...]
  6.   (interleaved with 5) o_part row tiles = repT2 K=128 matmuls against
       host-row-interleaved W_o^T slice; output rows at F(hp)*512 + r0*128 with
       F(hp) = 8*(hp>>3) + 4*(hp&1) + ((hp>>1)&3); host unscrambles
       (h', r0, b, hi) -> s = hi*64 + r0*16 + h'.
"""
import sys
import numpy as np

try:
    import concourse.bass  # noqa: F401
except ImportError:
    sys.path.insert(0, "/opt/trn_rl_repo")

B, S, D, H, HD = 4, 2048, 1024, 16, 64

_CACHE = {}


def _build_program():
    from contextlib import ExitStack

    import concourse.mybir as mybir
    import concourse.tile as tile
    from concourse import bacc

    F16 = mybir.dt.float16
    F32 = mybir.dt.float32
    AFT = mybir.ActivationFunctionType

    nc = bacc.Bacc(None, target_bir_lowering=False, debug=False)

    with tile.TileContext(nc) as tc:
        with tc.tile_pool(name="dram", bufs=1, space="DRAM") as dram:
            kT = dram.tile([1024, 2048], F16, kind="ExternalInput", name="kT", uniquify=False)
            qT = dram.tile([1024, 1024], F16, kind="ExternalInput", name="qT", uniquify=False)
            vT = dram.tile([1024, 2048], F16, kind="ExternalInput", name="vT", uniquify=False)
            wkT = dram.tile([1024, 1024], F16, kind="ExternalInput", name="wkT", uniquify=False)
            wqT = dram.tile([1024, 1024], F16, kind="ExternalInput", name="wqT", uniquify=False)
            wvT = dram.tile([1024, 1024], F16, kind="ExternalInput", name="wvT", uniquify=False)
            woTa = dram.tile([128, 1024], F16, kind="ExternalInput", name="woTa", uniquify=False)
            ones1 = dram.tile([128, 4], F16, kind="ExternalInput", name="ones1", uniquify=False)
            ident = dram.tile([128, 128], F16, kind="ExternalInput", name="ident", uniquify=False)
            o_part = dram.tile([8192, 1024], F16, kind="ExternalOutput", name="o_part", uniquify=False)

            # rotate psum evictions between DVE and ACT to balance engine load
            _rot = [0]

            def evict_copy(dst, src):
                _rot[0] += 1
                if _rot[0] & 1:
                    nc.vector.tensor_copy(dst, src)
                else:
                    nc.scalar.copy(dst, src)

            # W chunks go on the ACT HWDGE queue; x blocks on the SP HWDGE
            # queue so neither stream head-of-line-blocks the other. 256-col
            # fp16 chunks keep DMA descriptors at 512B (full bus rate).
            def load_w_full(pool, w_dram, nm):
                w_sb = pool.tile([128, 8, 1024], F16, name=nm, tag="wfull")
                w_r = w_dram.rearrange("(t p) c -> p t c", p=128)
                for cc in range(4):
                    nc.scalar.dma_start(w_sb[:, :, cc * 256:(cc + 1) * 256],
                                        w_r[:, :, cc * 256:(cc + 1) * 256])
                return w_sb

            def scatter_evict(dst_fn, ps, j, gcol0, width, nsig):
                seg = min(nsig, width)
                for s_off in range(0, width, seg):
                    gcol = gcol0 + s_off
                    b = gcol // nsig
                    tau = (j & 1) * 4 + b
                    c0 = (j >> 1) * nsig + (gcol % nsig)
                    evict_copy(dst_fn(tau)[:, c0:c0 + seg], ps[:, s_off:s_off + seg])

            def proj_scatter(dst_fn, x_dram, nsig, blocks, w_sb, stg, psp,
                             first_per_t=False):
                """Project x window by W^T; merged scatter-evict into shuffled-
                transposed dst. blocks = list of (col0, width)."""
                x_r = x_dram.rearrange("(t p) c -> p t c", p=128)
                first = True
                for c0b, wb in blocks:
                    x_sb = stg.tile([128, 8, 512], F16, name="x_sb", tag="x_sb")
                    if first and first_per_t:
                        for t in range(8):
                            nc.sync.dma_start(x_sb[:, t:t + 1, 0:wb],
                                              x_r[:, t:t + 1, c0b:c0b + wb])
                    else:
                        nc.sync.dma_start(x_sb[:, :, 0:wb], x_r[:, :, c0b:c0b + wb])
                    first = False
                    for j in range(8):
                        ps = psp.tile([128, 512], F32, name="ps", tag="ps")
                        for t in range(8):
                            nc.tensor.matmul(ps[:, 0:wb], w_sb[:, t, j * 128:(j + 1) * 128],
                                             x_sb[:, t, 0:wb], start=(t == 0), stop=(t == 7))
                        scatter_evict(dst_fn, ps[:, 0:wb], j, c0b, wb, nsig)

            stkKQ = ExitStack()
            pK = stkKQ.enter_context(tc.tile_pool(name="pK", bufs=1))
            K2T = pK.tile([128, 8, 2048], F16, name="K2T")
            pQ = stkKQ.enter_context(tc.tile_pool(name="pQ", bufs=1))
            Q2T = pQ.tile([128, 8, 1024], F16, name="Q2T")

            # phase 1 + 2: K and Q projections (full-W tiles, double-buffered)
            with tc.tile_pool(name="pW", bufs=2) as pW, \
                 tc.tile_pool(name="stp", bufs=3) as stp, \
                 tc.tile_pool(name="pspA", bufs=8, space="PSUM") as pspA:
                w_k = load_w_full(pW, wkT, "w_k")
                proj_scatter(lambda tau: K2T[:, tau, :], kT, 512,
                             [(0, 512), (512, 512), (1024, 512), (1536, 512)],
                             w_sb=w_k, stg=stp, psp=pspA, first_per_t=True)
                w_q = load_w_full(pW, wqT, "w_q")
                proj_scatter(lambda tau: Q2T[:, tau, :], qT, 256,
                             [(0, 512), (512, 512)], w_sb=w_q, stg=stp, psp=pspA)

            # phase 3: scores^T + exp
            stkE = ExitStack()
            pE = stkE.enter_context(tc.tile_pool(name="pE", bufs=1, side="right"))
            expS = pE.tile([128, 16, 1024], F16, name="expS")
            with tc.tile_pool(name="ps_s", bufs=8, space="PSUM") as pss:
                for qb in range(2):
                    for kt in range(16):
                        ps = pss.tile([128, 512], F32, name="ps_sc", tag="ps_sc")
                        for t in range(8):
                            nc.tensor.matmul(ps[:], K2T[:, t, kt * 128:(kt + 1) * 128],
                                             Q2T[:, t, qb * 512:(qb + 1) * 512],
                                             start=(t == 0), stop=(t == 7))
                        nc.scalar.activation(expS[:, kt, qb * 512:(qb + 1) * 512], ps[:],
                                             AFT.Exp, scale=1.0 / 32.0)
            stkKQ.close()

            # phase 4: V projection -> V''T -> PE-transpose -> V'' natural
            stkV = ExitStack()
            pV = stkV.enter_context(tc.tile_pool(name="pV", bufs=1))
            V2 = pV.tile([128, 16, 1024], F16, name="V2")
            stkI = ExitStack()
            cpool = stkI.enter_context(tc.tile_pool(name="cpool", bufs=1, side="right"))
            id_sb = cpool.tile([128, 128], F16, name="id_sb")
            nc.scalar.dma_start(id_sb[:], ident[:])
            with ExitStack() as ctxv:
                v2t_pool = ctxv.enter_context(tc.tile_pool(name="v2t", bufs=4))
                vstg = ctxv.enter_context(tc.tile_pool(name="vstg", bufs=4))
                wvp = ctxv.enter_context(tc.tile_pool(name="wvp", bufs=1))
                pspV = ctxv.enter_context(tc.tile_pool(name="pspV", bufs=4, space="PSUM"))
                pst_pool = ctxv.enter_context(tc.tile_pool(name="ps_t", bufs=3, space="PSUM"))
                w_v = load_w_full(wvp, wvT, "w_v")
                v2t_tiles = {}

                def v_dst(tau):
                    if tau not in v2t_tiles:
                        v2t_tiles[tau] = v2t_pool.tile([128, 2048], F16,
                                                       name=f"v2t_{tau}", tag="v2t")
                    return v2t_tiles[tau]

                v_r = vT.rearrange("(t p) c -> p t c", p=128)
                for pair in range(2):
                    x_pair = []
                    for bb in (2 * pair, 2 * pair + 1):
                        x_sb = vstg.tile([128, 8, 512], F16, name="x_sb", tag="vx_sb")
                        nc.sync.dma_start(x_sb[:], v_r[:, :, bb * 512:(bb + 1) * 512])
                        x_pair.append(x_sb)
                    for j in range(8):
                        for bi, bb in enumerate((2 * pair, 2 * pair + 1)):
                            ps = pspV.tile([128, 512], F32, name="ps", tag="vps")
                            for t in range(8):
                                nc.tensor.matmul(ps[:], w_v[:, t, j * 128:(j + 1) * 128],
                                                 x_pair[bi][:, t, :],
                                                 start=(t == 0), stop=(t == 7))
                            scatter_evict(v_dst, ps[:], j, bb * 512, 512, 512)
                    # pair 0 completes taus {0,1,4,5} (b in {0,1}); pair 1 the rest
                    taus = (0, 1, 4, 5) if pair == 0 else (2, 3, 6, 7)
                    for tau in taus:
                        vt = v2t_tiles.pop(tau)
                        for ktg in range(0, 16, 4):
                            pst = pst_pool.tile([128, 4, 128], F16, name="pst", tag="pst")
                            for ki in range(4):
                                nc.tensor.transpose(pst[:, ki, :],
                                                    vt[:, (ktg + ki) * 128:(ktg + ki + 1) * 128],
                                                    id_sb[:])
                            evict_copy(V2[:, ktg:ktg + 4, tau * 128:(tau + 1) * 128], pst[:])

            # phase 5: Z + AV -> rep (SBUF) -> PE-transpose -> repT (SBUF)
            stkR = ExitStack()
            pR = stkR.enter_context(tc.tile_pool(name="pR", bufs=1))
            repT2 = pR.tile([128, 16, 4, 128], F16, name="repT2")
            repp = stkR.enter_context(tc.tile_pool(name="repp", bufs=2))
            ostp = stkR.enter_context(tc.tile_pool(name="ostp", bufs=4))
            wop = stkR.enter_context(tc.tile_pool(name="wop", bufs=1))
            wo_a = wop.tile([128, 1024], F16, name="wo_a")
            nc.scalar.dma_start(wo_a[:], woTa[:])
            with ExitStack() as ctxa:
                cp2 = ctxa.enter_context(tc.tile_pool(name="cp2", bufs=1))
                ones_sb = cp2.tile([128, 4], F16, name="ones_sb")
                nc.scalar.dma_start(ones_sb[:], ones1[:])
                rzp = ctxa.enter_context(tc.tile_pool(name="rzp", bufs=2))
                pz = ctxa.enter_context(tc.tile_pool(name="pz", bufs=1, space="PSUM"))
                pav = ctxa.enter_context(tc.tile_pool(name="pav", bufs=2, space="PSUM"))
                prt = ctxa.enter_context(tc.tile_pool(name="prt", bufs=2, space="PSUM"))
                pso = ctxa.enter_context(tc.tile_pool(name="pso", bufs=3, space="PSUM"))

                def emit_phase6_r0(r0, h_lo=0, h_hi=16):
                    # needs repT2[:, :, r0, :] = qs subtiles 2r0 (par=0), 2r0+1 (par=1)
                    for hp16 in range(h_lo, h_hi):
                        ost = ostp.tile([128, 1024], F16, name="ost", tag="ost")
                        lhsT = repT2[:, hp16, r0, :]
                        fv = 8 * (hp16 >> 3) + 4 * (hp16 & 1) + ((hp16 >> 1) & 3)
                        row0 = fv * 512 + r0 * 128
                        for half in range(2):
                            po = pso.tile([128, 512], F32, name="po", tag="po")
                            nc.tensor.matmul(po[:], lhsT, wo_a[:, half * 512:(half + 1) * 512],
                                             start=True, stop=True)
                            evict_copy(ost[:, half * 512:(half + 1) * 512], po[:])
                            nc.sync.dma_start(
                                o_part[row0:row0 + 128, half * 512:(half + 1) * 512],
                                ost[:, half * 512:(half + 1) * 512])

                pairs, pending = {}, None

                def emit_pair_transposes(r0q):
                    rp = pairs.pop(r0q)
                    for hp0 in range(0, 16, 4):
                        prt_t = prt.tile([128, 4, 128], F16, name="prt_t", tag="prt_t")
                        for i in range(4):
                            nc.tensor.transpose(prt_t[:, i, :],
                                                rp[:, (hp0 + i) * 128:(hp0 + i + 1) * 128],
                                                id_sb[:])
                        evict_copy(repT2[:, hp0:hp0 + 4, r0q, :], prt_t[:])

                for qs in range(8):
                    if qs % 2 == 0 and pending is not None:
                        emit_phase6_r0(pending, 0, 4)
                    zp = pz.tile([128, 4], F32, name="zp", tag="zp")
                    for kt in range(16):
                        nc.tensor.matmul(zp[:], expS[:, kt, qs * 128:(qs + 1) * 128],
                                         ones_sb[:], start=(kt == 0), stop=(kt == 15))
                    rz = rzp.tile([128, 1], F32, name="rz", tag="rz")
                    nc.vector.reciprocal(rz[:], zp[:, 0:1])
                    par, r0q = qs & 1, qs >> 1
                    if par == 0:
                        pairs[r0q] = repp.tile([128, 2048], F16, name="rep_pair", tag="repx")
                    rep_pair = pairs[r0q]
                    for df in range(2):
                        pa = pav.tile([128, 512], F32, name="pa", tag="pa")
                        for kt in range(16):
                            nc.tensor.matmul(pa[:], expS[:, kt, qs * 128:(qs + 1) * 128],
                                             V2[:, kt, df * 512:(df + 1) * 512],
                                             start=(kt == 0), stop=(kt == 15))
                        # interleaved dest: col = d*2 + parity
                        nc.scalar.activation(
                            rep_pair[:, df * 1024 + par:df * 1024 + par + 1023:2], pa[:],
                            AFT.Copy, scale=rz[:])
                        if df == 0 and par == 0 and pending is not None:
                            emit_phase6_r0(pending, 4, 10)
                    if par == 0 and pending is not None:
                        emit_phase6_r0(pending, 10, 16)
                        pending = None
                    if par == 1:
                        emit_pair_transposes(r0q)
                        pending = r0q
                emit_phase6_r0(pending)
            stkI.close()
            stkE.close()

            stkR.close()
            stkV.close()

    nc.compile()
    return nc


def _host_inputs(k, q, v, W_k, W_q, W_v, W_o):
    """Per-core input maps. Core c: bp = c//2 (shuffled batch), qh = c%2."""
    f16 = np.float16
    W_kT = np.ascontiguousarray(W_k.T, dtype=f16)
    W_qT = np.ascontiguousarray(W_q.T, dtype=f16)
    W_vT = np.ascontiguousarray(W_v.T, dtype=f16)
    W_oT = np.ascontiguousarray(W_o.T, dtype=np.float32)
    ones = np.ones((128, 4), dtype=f16)
    ident = np.eye(128, dtype=f16)
    in_maps = []
    for c in range(8):
        bp, qh = c // 2, c % 2
        kw = k[:, 512 * bp:512 * (bp + 1), :].reshape(2048, 1024)
        vw = v[:, 512 * bp:512 * (bp + 1), :].reshape(2048, 1024)
        qw = q[:, 512 * bp + 256 * qh:512 * bp + 256 * (qh + 1), :].reshape(1024, 1024)
        h0 = 4 * bp + 2 * qh
        wo_nat = W_oT[h0 * 64:h0 * 64 + 128, :]
        wo_nat = np.ascontiguousarray(
            wo_nat.reshape(2, 64, 1024).transpose(1, 0, 2).reshape(128, 1024), dtype=f16)
        in_maps.append({
            "kT": np.ascontiguousarray(kw.T, dtype=f16),
            "vT": np.ascontiguousarray(vw.T, dtype=f16),
            "qT": np.ascontiguousarray(qw.T, dtype=f16),
            "wkT": W_kT, "wqT": W_qT, "wvT": W_vT,
            "woTa": wo_nat,
            "ones1": ones, "ident": ident,
        })
    return in_maps


def kernel(k, q, v, W_k, W_q, W_v, W_o, _want_trace=False):
    from concourse.bass_utils import run_bass_kernel_spmd

    if "nc" not in _CACHE:
        _CACHE["nc"] = _build_program()
    nc = _CACHE["nc"]

    in_maps = _host_inputs(np.asarray(k), np.asarray(q), np.asarray(v),
                           np.asarray(W_k), np.asarray(W_q), np.asarray(W_v),
                           np.asarray(W_o))
    res = run_bass_kernel_spmd(nc, in_maps, core_ids=list(range(8)),
                               trace=_want_trace)
    out = np.zeros((8192, 1024), dtype=np.float32)
    for r in res.results:
        out += r["o_part"].astype(np.float32)
    # rows are (h', r0, b, hi); real s = hi*64 + r0*16 + h'
    out = out.reshape(16, 4, 4, 32, D).transpose(2, 3, 1, 0, 4).reshape(B, S, D)
    if _want_trace:
        _CACHE["last_result"] = res
    return out
